# revision 1
# baseline (speedup 1.0000x reference)
"""Multi-scale deformable attention on 8 Trainium2 NeuronCores (Bass/Tile).

Sharding: core c = (batch b = c//4, head-pair hg = c%4) — each core handles 2
of the 8 heads for one batch element, all 10000 queries. Host sums the 4
partial output projections per batch and adds b_out.

Per-core pipeline:
  1. PE: value projection with 4 pixel-shifts (0, 1, W, W+1) -> "quad rows"
     [v(y0,x0), v(y0,x0+1), v(y1,x0), v(y1,x1)] per head per pixel, stored to
     DRAM as [NQROWS, 128] f32 (512B rows).
  2. PE: offset / attention projections from query_T; softmax via ACT exp +
     PE indicator-sum + DVE reciprocal.
  3. DVE: sampling grid -> clamped corner + hat weights (handles all edge /
     zero-padding cases exactly), fused attn*recip weights u, int16 row idx.
  4. Loop: SWDGE dma_gather of 512B quad rows (partition = (qlo,s) sample
     slot), DVE broadcast-weight multiply + corner reduce, PE indicator
     matmul reduces 16 (level,point) partitions -> per-(q,head) accumulators.
  5. PE: output projection W_out slice; host sums head-pair partials.
"""

import os

import numpy as np

import concourse.bass as bass
import concourse.bacc as bacc
import concourse.tile as tile
from concourse import mybir
from concourse import bass_utils

F32 = mybir.dt.float32
I32 = mybir.dt.int32
I16 = mybir.dt.int16
ALU = mybir.AluOpType
ACTF = mybir.ActivationFunctionType

# ---- problem constants (hardcoded; must match reference) ----
EMBED = 256
HEADS = 8
LEVELS = 4
POINTS = 4
D = 32
SHAPES = [(100, 100), (50, 50), (25, 25), (13, 13)]
HWS = [h * w for h, w in SHAPES]          # [10000, 2500, 625, 169]
NV = sum(HWS)                              # 13294
BS = 2
NQ = 10000

# quad-row (padded) level layout, per head
REG = [((hw + 127) // 128) * 128 for hw in HWS]   # [10112, 2560, 640, 256]
LB = [0]
for _r in REG[:-1]:
    LB.append(LB[-1] + _r)                # [0, 10112, 12672, 13312]
HEADREG = sum(REG)                         # 13568
NQROWS = 2 * HEADREG                       # 27136
VLB = [0]
for _hw in HWS[:-1]:
    VLB.append(VLB[-1] + _hw)             # [0, 10000, 12500, 13125]
VT_COLS = 13440                            # padded value_T cols

NSLOT = 2500                               # g slots (q = 4g + qlo)
G = 48                                     # slots per gather chunk
NCHUNK = 53
NSLOT_PAD = G * NCHUNK                     # 2544
T16_COLS = NSLOT_PAD * 8                   # 20352
QCH = 512                                  # q-column chunk for PE phases
NQCH = (NQ + QCH - 1) // QCH               # 20
PH4CH = 636                                # phase-4 column chunk

_CACHE = {}


def A(t, part, dims, off=0):
    """AP over tile/tensor `t`: part=(start, count, step) in rows/partitions,
    dims=[[step, count], ...] flat-element steps, off = extra element offset."""
    a = t if isinstance(t, bass.AP) else t[:]
    pitch = a.ap[0][0]
    start, cnt, pstep = part
    return bass.AP(tensor=a.tensor, offset=a.offset + start * pitch + off,
                   ap=[[pstep * pitch, cnt]] + [list(d) for d in dims])


def RAW(t, dims, off=0):
    a = t if isinstance(t, bass.AP) else t[:]
    return bass.AP(tensor=a.tensor, offset=a.offset + off,
                   ap=[list(d) for d in dims])


def build_nc(debug=False):
    KPHASE = int(os.environ.get("KPHASE", "9"))
    nc = bacc.Bacc("TRN2", target_bir_lowering=False, debug=False, num_devices=8)

    def din(name, shape, dt=F32):
        return nc.dram_tensor(name, shape, dt, kind="ExternalInput").ap()

    def dout(name, shape, dt=F32):
        return nc.dram_tensor(name, shape, dt, kind="ExternalOutput").ap()

    def dx(name, shape, dt=F32):
        kind = "ExternalOutput" if debug else "Internal"
        return nc.dram_tensor(name, shape, dt, kind=kind).ap()

    vT = din("vT", [256, VT_COLS])
    qT = din("qT", [256, NQ])
    refx = din("refx", [128, NSLOT_PAD])
    refy = din("refy", [128, NSLOT_PAD])
    wvT = din("wvT", [256, 64])
    woffT = din("woffT", [256, 64])
    wattnT = din("wattnT", [256, 32])
    battn = din("battn", [32, 1])
    bvalrep = din("bvalrep", [128, 64])
    woutT = din("woutT", [64, 256])
    ind8 = din("ind8", [128, 8])
    indsum = din("indsum", [32, 2])
    wcol = din("wcol", [128, 1])
    w2col = din("w2col", [128, 1])
    h2col = din("h2col", [128, 1])
    basecol = din("basecol", [128, 1])

    out_T = dout("out_T", [256, NQ])

    quad_dram = nc.dram_tensor("quad_dram", [NQROWS, 128], F32, kind="Internal").ap()
    wrap16 = nc.dram_tensor("wrap16", [16, T16_COLS], I16, kind="Internal").ap()
    stage_dram = nc.dram_tensor("stage_dram", [8, NSLOT_PAD * 32], F32,
                                kind="Internal").ap()
    off_dram = dx("off_dram", [64, NQ])
    attn_dram = dx("attn_dram", [32, NQ])
    sums_dram = dx("sums_dram", [2, NQ])

    dbg = {}
    if debug:
        for nm, shp, dt in [
            ("d_gx", [128, NSLOT_PAD], F32), ("d_gy", [128, NSLOT_PAD], F32),
            ("d_idx", [128, NSLOT_PAD], I16), ("d_u", [128, NSLOT_PAD, 4], F32),
            ("d_acc", [64, NQ], F32),
            ("d_red1", [128, G * 32], F32), ("d_gt", [128, G * 128], F32),
        ]:
            dbg[nm] = dout(nm, shp, dt)

    def _body():
        with tile.TileContext(nc) as tc:
          with tc.tile_pool(name="consts", bufs=1) as consts, \
               tc.tile_pool(name="persist", bufs=1) as persist:
            # persistent across the gather loop
            u_t = persist.tile([128, NSLOT_PAD, 4], F32)
            idx16 = persist.tile([128, NSLOT_PAD], I16)

            wv_t = consts.tile([128, 2, 64], F32)
            nc.sync.dma_start(out=wv_t[:],
                              in_=A(wvT, (0, 128, 1), [[128 * 64, 2], [1, 64]]))
            bval_t = consts.tile([128, 64], F32)
            nc.sync.dma_start(out=bval_t[:], in_=bvalrep[:])
            battn_t = consts.tile([32, 1], F32)
            nc.sync.dma_start(out=battn_t[:], in_=battn[:])
            indsum_t = consts.tile([32, 2], F32)
            nc.sync.dma_start(out=indsum_t[:], in_=indsum[:])
            ind8_t = consts.tile([128, 8], F32)
            nc.sync.dma_start(out=ind8_t[:], in_=ind8[:])
            woff_t = consts.tile([128, 2, 64], F32)
            nc.sync.dma_start(out=woff_t[:],
                              in_=A(woffT, (0, 128, 1), [[128 * 64, 2], [1, 64]]))
            wattn_t = consts.tile([128, 2, 32], F32)
            nc.sync.dma_start(out=wattn_t[:],
                              in_=A(wattnT, (0, 128, 1), [[128 * 32, 2], [1, 32]]))
            wcol_t = consts.tile([128, 1], F32)
            w2col_t = consts.tile([128, 1], F32)
            h2col_t = consts.tile([128, 1], F32)
            basecol_t = consts.tile([128, 1], F32)
            nc.sync.dma_start(out=wcol_t[:], in_=wcol[:])
            nc.sync.dma_start(out=w2col_t[:], in_=w2col[:])
            nc.sync.dma_start(out=h2col_t[:], in_=h2col[:])
            nc.sync.dma_start(out=basecol_t[:], in_=basecol[:])
            wout_t = consts.tile([64, 256], F32)
            nc.sync.dma_start(out=wout_t[:], in_=woutT[:])
            two_col = consts.tile([128, 1], F32)
            nc.vector.memset(two_col[:], 2.0)

            # ---------------- phase 1+2: value proj -> quad rows -------------
            GRP = 12  # pixel-chunks per DMA group
            with tc.tile_pool(name="vt", bufs=1) as vt_pool, \
                 tc.tile_pool(name="gb", bufs=2) as gb_pool, \
                 tc.tile_pool(name="qps", bufs=4, space="PSUM") as qps_pool:
                vt = vt_pool.tile([128, 2, VT_COLS], F32)
                nc.sync.dma_start(
                    out=vt[:],
                    in_=A(vT, (0, 128, 1), [[128 * VT_COLS, 2], [1, VT_COLS]]))
                for lvl in range(LEVELS):
                    Hl, Wl = SHAPES[lvl]
                    nch = (HWS[lvl] + 127) // 128
                    for g0 in range(0, nch, GRP):
                        ng = min(GRP, nch - g0)
                        gb = gb_pool.tile([128, GRP, 256], F32)
                        for j in range(ng):
                            pix = (g0 + j) * 128
                            ps = qps_pool.tile([128, 256], F32)
                            for si, shift in enumerate([0, 1, Wl, Wl + 1]):
                                col = VLB[lvl] + pix + shift
                                for k in range(2):
                                    nc.tensor.matmul(
                                        ps[:, si * 64:(si + 1) * 64],
                                        vt[:, k, col:col + 128],
                                        wv_t[:, k, :],
                                        start=(k == 0), stop=(k == 1))
                            # psum cols (shift, h, d) -> gb cols (h, shift, d) + bias
                            nc.vector.tensor_tensor(
                                A(gb, (0, 128, 1), [[128, 2], [32, 4], [1, 32]],
                                  off=j * 256),
                                A(ps, (0, 128, 1), [[32, 2], [64, 4], [1, 32]]),
                                A(bval_t, (0, 128, 1), [[32, 2], [0, 4], [1, 32]]),
                                ALU.add)
                        for h in range(2):
                            row0 = h * HEADREG + LB[lvl] + g0 * 128
                            nc.sync.dma_start(
                                out=A(quad_dram, (row0, 128, 1),
                                      [[128 * 128, ng], [1, 128]]),
                                in_=A(gb, (0, 128, 1), [[256, ng], [1, 128]],
                                      off=h * 128))

            # ---------------- phase 3: off / attn / softmax -> DRAM -----------
            if KPHASE < 2:
                return
            with tc.tile_pool(name="ph3", bufs=3) as ph3, \
                 tc.tile_pool(name="qt", bufs=3) as qt_pool, \
                 tc.tile_pool(name="p3ps", bufs=2, space="PSUM") as p3ps:
                for qc in range(NQCH):
                    q0 = qc * QCH
                    n = min(QCH, NQ - q0)
                    qt = qt_pool.tile([128, 2, QCH], F32)
                    nc.sync.dma_start(
                        out=qt[:, :, :n],
                        in_=A(qT, (0, 128, 1), [[128 * NQ, 2], [1, n]], off=q0))
                    pso = p3ps.tile([64, QCH], F32)
                    psa = p3ps.tile([32, QCH], F32)
                    for k in range(2):
                        nc.tensor.matmul(pso[:, :n], woff_t[:, k, :], qt[:, k, :n],
                                         start=(k == 0), stop=(k == 1))
                    for k in range(2):
                        nc.tensor.matmul(psa[:, :n], wattn_t[:, k, :], qt[:, k, :n],
                                         start=(k == 0), stop=(k == 1))
                    osb = ph3.tile([64, QCH], F32, tag="osb")
                    asb = ph3.tile([32, QCH], F32, tag="asb")
                    ssb = ph3.tile([2, QCH], F32, tag="ssb")
                    nc.scalar.activation(osb[:, :n], pso[:, :n], ACTF.Copy)
                    nc.scalar.activation(asb[:, :n], psa[:, :n], ACTF.Exp,
                                         bias=battn_t[:])
                    pss = p3ps.tile([2, QCH], F32)
                    nc.tensor.matmul(pss[:, :n], indsum_t[:], asb[:, :n],
                                     start=True, stop=True)
                    nc.scalar.activation(ssb[:, :n], pss[:, :n], ACTF.Copy)
                    nc.sync.dma_start(out=off_dram[:, q0:q0 + n], in_=osb[:, :n])
                    nc.sync.dma_start(out=attn_dram[:, q0:q0 + n], in_=asb[:, :n])
                    nc.sync.dma_start(out=sums_dram[:, q0:q0 + n], in_=ssb[:, :n])

            # ------- phase 3.5: rearrange to 128-partition layout -------------
            if KPHASE < 3:
                return
            with tc.tile_pool(name="lay", bufs=1) as lay:
                offx = lay.tile([128, NSLOT_PAD], F32)
                offy = lay.tile([128, NSLOT_PAD], F32)
                attn1 = lay.tile([128, NSLOT_PAD], F32)
                sums1 = lay.tile([128, NSLOT_PAD], F32)
                nc.vector.memset(offx[:], 0.0)
                nc.vector.memset(offy[:], 0.0)
                nc.vector.memset(attn1[:], 0.0)
                nc.vector.memset(sums1[:], 1.0)
                for qlo in range(4):
                    nc.sync.dma_start(
                        out=A(offx, (qlo * 32, 32, 1), [[1, NSLOT]]),
                        in_=A(off_dram, (0, 32, 2), [[4, NSLOT]], off=qlo))
                    nc.sync.dma_start(
                        out=A(offy, (qlo * 32, 32, 1), [[1, NSLOT]]),
                        in_=A(off_dram, (1, 32, 2), [[4, NSLOT]], off=qlo))
                    # split into 8-row groups: a single DMA would coalesce to an
                    # 80000-element dim (> 16-bit ISA field)
                    for rg in range(4):
                        nc.sync.dma_start(
                            out=A(attn1, (qlo * 32 + rg * 8, 8, 1), [[1, NSLOT]]),
                            in_=A(attn_dram, (rg * 8, 8, 1), [[4, NSLOT]], off=qlo))
                    for h in range(2):
                        nc.sync.dma_start(
                            out=A(sums1, (qlo * 32 + h * 16, 16, 1), [[1, NSLOT]]),
                            in_=RAW(sums_dram, [[0, 16], [4, NSLOT]],
                                    off=h * NQ + qlo))

                # -------- phase 4: weights + indices (column-chunked) ---------
                with tc.tile_pool(name="p4", bufs=2) as p4:
                    for ci in range(NSLOT_PAD // PH4CH):
                        c0 = ci * PH4CH
                        CW = PH4CH
                        SH = [128, CW]
                        sl = slice(c0, c0 + CW)

                        rfx = p4.tile(SH, F32, tag="rfx")
                        rfy = p4.tile(SH, F32, tag="rfy")
                        nc.sync.dma_start(out=rfx[:], in_=refx[:, sl])
                        nc.sync.dma_start(out=rfy[:], in_=refy[:, sl])
                        gxt = p4.tile(SH, F32, tag="gx")
                        gyt = p4.tile(SH, F32, tag="gy")
                        nc.vector.tensor_tensor(gxt[:], offx[:, sl], rfx[:], ALU.add)
                        nc.vector.tensor_tensor(gyt[:], offy[:, sl], rfy[:], ALU.add)
                        rcp = p4.tile(SH, F32, tag="rcp")
                        nc.vector.reciprocal(rcp[:], sums1[:, sl])

                        tt = p4.tile(SH, F32, tag="tt")
                        t2 = p4.tile(SH, F32, tag="t2")
                        ti = p4.tile(SH, I32, tag="ti")
                        x0c = p4.tile(SH, F32, tag="x0c")
                        y0c = p4.tile(SH, F32, tag="y0c")
                        # floor(g): c = int(g - 0.5) (round OR trunc semantics both
                        # give c in {floor-1, floor} for g>0); fix up with
                        # +1 if g - c >= 1. Negative g cases are absorbed by the
                        # clamp + hat-weight formulation.
                        for g_t, cl_t, out_t in ((gxt, w2col_t, x0c),
                                                 (gyt, h2col_t, y0c)):
                            nc.vector.tensor_scalar(tt[:], g_t[:], 0.5, None,
                                                    ALU.subtract)
                            nc.vector.tensor_copy(ti[:], tt[:])
                            nc.vector.tensor_copy(tt[:], ti[:])
                            nc.vector.tensor_tensor(t2[:], g_t[:], tt[:], ALU.subtract)
                            nc.vector.tensor_scalar(t2[:], t2[:], 1.0, None, ALU.is_ge)
                            nc.vector.tensor_tensor(tt[:], tt[:], t2[:], ALU.add)
                            nc.vector.tensor_scalar(out_t[:], tt[:], 0.0, cl_t[:],
                                                    ALU.max, ALU.min)

                        # hat weights: hat(d) = min(relu(1-d), relu(1+d)),
                        # shifted hat: hat(d-1) = min(relu(d), relu(2-d))
                        dxt = p4.tile(SH, F32, tag="dxt")
                        dyt = p4.tile(SH, F32, tag="dyt")
                        wx0 = p4.tile(SH, F32, tag="wx0")
                        wx1 = p4.tile(SH, F32, tag="wx1")
                        wy0 = p4.tile(SH, F32, tag="wy0")
                        wy1 = p4.tile(SH, F32, tag="wy1")
                        ra = p4.tile(SH, F32, tag="ra")
                        nc.vector.tensor_tensor(dxt[:], gxt[:], x0c[:], ALU.subtract)
                        nc.scalar.activation(ra[:], dxt[:], ACTF.Relu,
                                             bias=1.0, scale=-1.0)
                        nc.scalar.activation(tt[:], dxt[:], ACTF.Relu,
                                             bias=1.0, scale=1.0)
                        nc.vector.tensor_tensor(wx0[:], ra[:], tt[:], ALU.min)
                        nc.scalar.activation(ra[:], dxt[:], ACTF.Relu,
                                             bias=0.0, scale=1.0)
                        nc.scalar.activation(tt[:], dxt[:], ACTF.Relu,
                                             bias=two_col[:], scale=-1.0)
                        nc.vector.tensor_tensor(wx1[:], ra[:], tt[:], ALU.min)
                        nc.vector.tensor_tensor(dyt[:], gyt[:], y0c[:], ALU.subtract)
                        nc.scalar.activation(ra[:], dyt[:], ACTF.Relu,
                                             bias=1.0, scale=-1.0)
                        nc.scalar.activation(tt[:], dyt[:], ACTF.Relu,
                                             bias=1.0, scale=1.0)
                        nc.vector.tensor_tensor(wy0[:], ra[:], tt[:], ALU.min)
                        nc.scalar.activation(ra[:], dyt[:], ACTF.Relu,
                                             bias=0.0, scale=1.0)
                        nc.scalar.activation(tt[:], dyt[:], ACTF.Relu,
                                             bias=two_col[:], scale=-1.0)
                        nc.vector.tensor_tensor(wy1[:], ra[:], tt[:], ALU.min)

                        ubt = p4.tile(SH, F32, tag="ubt")
                        nc.vector.tensor_tensor(ubt[:], attn1[:, sl], rcp[:], ALU.mult)
                        uy0 = p4.tile(SH, F32, tag="uy0")
                        uy1 = p4.tile(SH, F32, tag="uy1")
                        nc.vector.tensor_tensor(uy0[:], ubt[:], wy0[:], ALU.mult)
                        nc.vector.tensor_tensor(uy1[:], ubt[:], wy1[:], ALU.mult)
                        for c, (uy, wx) in enumerate([(uy0, wx0), (uy0, wx1),
                                                      (uy1, wx0), (uy1, wx1)]):
                            nc.vector.tensor_tensor(
                                A(u_t, (0, 128, 1), [[4, CW]], off=c0 * 4 + c),
                                uy[:], wx[:], ALU.mult)

                        nc.vector.tensor_scalar(tt[:], y0c[:], wcol_t[:],
                                                basecol_t[:], ALU.mult, ALU.add)
                        nc.vector.tensor_tensor(tt[:], tt[:], x0c[:], ALU.add)
                        nc.vector.tensor_copy(idx16[:, sl], tt[:])

                        if debug:
                            nc.sync.dma_start(out=dbg["d_gx"][:, sl], in_=gxt[:])
                            nc.sync.dma_start(out=dbg["d_gy"][:, sl], in_=gyt[:])

            if debug:
                nc.sync.dma_start(out=dbg["d_idx"][:], in_=idx16[:])
                nc.sync.dma_start(out=dbg["d_u"][:], in_=u_t[:])

            # wrap idx into the dma_gather 16-partition stream layout (via DRAM)
            if KPHASE < 4:
                return
            for qlo in range(4):
                for sh in range(2):
                    nc.sync.dma_start(
                        out=A(wrap16, (0, 16, 1), [[8, NSLOT_PAD]], off=qlo * 2 + sh),
                        in_=A(idx16, (qlo * 32 + sh * 16, 16, 1), [[1, NSLOT_PAD]]))

            # ---------------- phase 6: gather + weighted reduce ---------------
            if KPHASE < 5:
                return
            with tc.tile_pool(name="gl", bufs=1) as gl, \
                 tc.tile_pool(name="gt", bufs=2) as gt_pool, \
                 tc.tile_pool(name="prod", bufs=1) as prod_pool, \
                 tc.tile_pool(name="red", bufs=2) as red_pool, \
                 tc.tile_pool(name="stg", bufs=2) as stg_pool, \
                 tc.tile_pool(name="gps", bufs=4, space="PSUM") as gps_pool:
                t16 = gl.tile([128, T16_COLS], I16)
                nc.sync.dma_start(
                    out=t16[:],
                    in_=RAW(wrap16, [[0, 8], [T16_COLS, 16], [1, T16_COLS]]))

                for c in range(NCHUNK):
                    gt = gt_pool.tile([128, G, 128], F32)
                    nc.gpsimd.dma_gather(
                        gt[:], quad_dram[:], t16[:, c * (G * 8):(c + 1) * (G * 8)],
                        G * 128, G * 128, 128, single_packet=False)
                    prod = prod_pool.tile([128, G * 128], F32)
                    nc.vector.tensor_tensor(
                        prod[:], gt[:],
                        A(u_t, (0, 128, 1), [[4, G], [1, 4], [0, 32]], off=c * G * 4),
                        ALU.mult)
                    red1 = red_pool.tile([128, G * 32], F32)
                    nc.vector.tensor_reduce(
                        red1[:],
                        A(prod, (0, 128, 1), [[128, G], [1, 32], [32, 4]]),
                        op=ALU.add, axis=mybir.AxisListType.X)
                    stg = stg_pool.tile([8, G * 32], F32)
                    for j in range(3):
                        ps = gps_pool.tile([8, QCH], F32)
                        nc.tensor.matmul(ps[:], ind8_t[:],
                                         red1[:, j * QCH:(j + 1) * QCH],
                                         start=True, stop=True)
                        nc.scalar.activation(stg[:, j * QCH:(j + 1) * QCH], ps[:],
                                             ACTF.Copy)
                    nc.sync.dma_start(
                        out=A(stage_dram, (0, 8, 1), [[1, G * 32]], off=c * G * 32),
                        in_=stg[:])
                    if debug and c == 0:
                        nc.sync.dma_start(out=dbg["d_gt"][:], in_=gt[:])
                        nc.sync.dma_start(out=dbg["d_red1"][:], in_=red1[:])

            # ---------------- phase 7: out projection -------------------------
            if KPHASE < 6:
                return
            with tc.tile_pool(name="ph7", bufs=1) as ph7, \
                 tc.tile_pool(name="p7ps", bufs=4, space="PSUM") as p7ps:
                acc_T = ph7.tile([64, NQ], F32)
                for qlo in range(4):
                    for h in range(2):
                        m = qlo * 2 + h
                        nc.sync.dma_start(
                            out=A(acc_T, (h * 32, 32, 1), [[4, NSLOT]], off=qlo),
                            in_=A(stage_dram, (m, 1, 1), [[1, 32], [32, NSLOT]]))
                if debug:
                    nc.sync.dma_start(out=dbg["d_acc"][:], in_=acc_T[:])
                osb = ph7.tile([128, 2, NQ], F32)
                for qc in range(NQCH):
                    q0 = qc * QCH
                    n = min(QCH, NQ - q0)
                    for mh in range(2):
                        ps = p7ps.tile([128, QCH], F32)
                        nc.tensor.matmul(ps[:, :n], wout_t[:, mh * 128:(mh + 1) * 128],
                                         acc_T[:, q0:q0 + n], start=True, stop=True)
                        nc.scalar.activation(osb[:, mh, q0:q0 + n], ps[:, :n], ACTF.Copy)
                nc.sync.dma_start(
                    out=A(out_T, (0, 128, 1), [[128 * NQ, 2], [1, NQ]]),
                    in_=osb[:])

    _body()
    nc.compile()
    return nc


def _prep_core_inputs(b, hg, query, value, reference_points,
                      W_off, b_off, W_attn, b_attn, W_val, b_val, W_out):
    """Host-side per-core input dict (all f32 numpy)."""
    f = np.float32
    vT = np.zeros((256, VT_COLS), f)
    vT[:, :NV] = value[b].T
    qT = np.ascontiguousarray(query[b].T.astype(f))

    s_arr = np.arange(32)
    h_loc = s_arr // 16
    l_arr = (s_arr // 4) % 4
    Wl = np.array([SHAPES[l][1] for l in l_arr], f)
    Hl = np.array([SHAPES[l][0] for l in l_arr], f)
    base = np.array([h_loc[s] * HEADREG + LB[l_arr[s]] for s in range(32)], f)

    boff = b_off[hg * 64:(hg + 1) * 64].astype(f)  # rows (s, xy)
    refx = np.zeros((128, NSLOT_PAD), f)
    refy = np.zeros((128, NSLOT_PAD), f)
    ref = np.asarray(reference_points)[b, :, 0, :].astype(f)  # [NQ, 2] (x, y)
    for qlo in range(4):
        rx = ref[4 * np.arange(NSLOT) + qlo, 0]
        ry = ref[4 * np.arange(NSLOT) + qlo, 1]
        for s in range(32):
            refx[qlo * 32 + s, :NSLOT] = rx * Wl[s] - 0.5 + boff[s * 2 + 0]
            refy[qlo * 32 + s, :NSLOT] = ry * Hl[s] - 0.5 + boff[s * 2 + 1]

    wvT = np.ascontiguousarray(W_val[hg * 64:(hg + 1) * 64, :].T.astype(f))
    woffT = np.ascontiguousarray(W_off[hg * 64:(hg + 1) * 64, :].T.astype(f))
    wattnT = np.ascontiguousarray(W_attn[hg * 32:(hg + 1) * 32, :].T.astype(f))
    battn = b_attn[hg * 32:(hg + 1) * 32].astype(f).reshape(32, 1)
    bvalrep = np.tile(b_val[hg * 64:(hg + 1) * 64].astype(f)[None, :], (128, 1))
    woutT = np.ascontiguousarray(W_out[:, hg * 64:(hg + 1) * 64].T.astype(f))

    p_arr = np.arange(128)
    ind8 = np.zeros((128, 8), f)
    ind8[p_arr, (p_arr // 32) * 2 + (p_arr % 32) // 16] = 1.0
    indsum = np.zeros((32, 2), f)
    indsum[np.arange(32), np.arange(32) // 16] = 1.0

    pcol = lambda v: np.ascontiguousarray(np.tile(v.astype(f), 4).reshape(128, 1))
    return {
        "vT": vT, "qT": qT, "refx": refx, "refy": refy,
        "wvT": wvT, "woffT": woffT, "wattnT": wattnT, "battn": battn,
        "bvalrep": bvalrep, "woutT": woutT, "ind8": ind8, "indsum": indsum,
        "wcol": pcol(Wl), "w2col": pcol(Wl - 2), "h2col": pcol(Hl - 2),
        "basecol": pcol(base),
    }


def run_cores(inputs, debug=False, trace=False):
    key = ("nc", debug)
    if key not in _CACHE:
        _CACHE[key] = build_nc(debug=debug)
    nc = _CACHE[key]
    in_maps = [_prep_core_inputs(c // 4, c % 4, **inputs) for c in range(8)]
    res = bass_utils.run_bass_kernel_spmd(nc, in_maps, core_ids=list(range(8)),
                                          trace=trace)
    return res


def kernel(query, value, reference_points, spatial_shapes,
           W_off, b_off, W_attn, b_attn, W_val, b_val, W_out, b_out,
           _debug=False, _trace=False):
    inputs = dict(query=np.asarray(query), value=np.asarray(value),
                  reference_points=np.asarray(reference_points),
                  W_off=np.asarray(W_off), b_off=np.asarray(b_off),
                  W_attn=np.asarray(W_attn), b_attn=np.asarray(b_attn),
                  W_val=np.asarray(W_val), b_val=np.asarray(b_val),
                  W_out=np.asarray(W_out))
    res = run_cores(inputs, debug=_debug, trace=_trace)
    out = np.zeros((BS, NQ, 256), np.float32)
    for b in range(BS):
        acc = np.zeros((256, NQ), np.float32)
        for hg in range(4):
            acc += res.results[b * 4 + hg]["out_T"]
        out[b] = acc.T + np.asarray(b_out)[None, :].astype(np.float32)
    kernel._last_res = res
    return out



# revision 2
# speedup vs baseline: 1.3151x; 1.3151x over previous
"""Multi-scale deformable attention on 8 Trainium2 NeuronCores (Bass/Tile).

Sharding: core c = (batch b = c//4, head-pair hg = c%4) — each core handles 2
of the 8 heads for one batch element, all 10000 queries. Host sums the 4
partial output projections per batch and adds b_out.

Per-core pipeline (ordered so the SWDGE gather desc-gen — the serial floor —
starts as early as possible):
  1. PE: offset / attention projections from query_T; softmax via ACT exp +
     PE indicator-sum + DVE reciprocal.  -> off/attn/sums in DRAM.
  2. Relayout to 128-partition (qblock, s) layout: q = qlo*2500 + g so every
     DMA run is contiguous (no 4-byte packets).
  3. DVE: sampling grid -> clamped corner + hat weights, fused attn*recip
     weights u, int16 row idx; wrap idx into the 16-row gather stream layout
     (stride-8 interleave DMAs split across the SP + ACT HWDGE queues).
  4. PE: value projection with 4 pixel-shifts (0, 1, W, W+1) -> "quad rows"
     [v(y0,x0), v(y0,x0+1), v(y1,x0), v(y1,x1)] per head per pixel, stored to
     DRAM as [NQROWS, 128] f32 (512B rows).
  5. Loop: SWDGE dma_gather of 512B quad rows (partition = (qlo,s) sample
     slot), DVE broadcast-weight multiply + corner reduce, PE indicator
     matmul reduces 16 (level,point) partitions -> per-(q,head) accumulators,
     staged to DRAM in (d, g) order so phase-7 loads are contiguous.
  6. PE: output projection W_out slice; host sums head-pair partials.
"""

import os

import numpy as np

import concourse.bass as bass
import concourse.bacc as bacc
import concourse.tile as tile
from concourse import mybir
from concourse import bass_utils

F32 = mybir.dt.float32
I32 = mybir.dt.int32
I16 = mybir.dt.int16
ALU = mybir.AluOpType
ACTF = mybir.ActivationFunctionType

# ---- problem constants (hardcoded; must match reference) ----
EMBED = 256
HEADS = 8
LEVELS = 4
POINTS = 4
D = 32
SHAPES = [(100, 100), (50, 50), (25, 25), (13, 13)]
HWS = [h * w for h, w in SHAPES]          # [10000, 2500, 625, 169]
NV = sum(HWS)                              # 13294
BS = 2
NQ = 10000

# quad-row (padded) level layout, per head
REG = [((hw + 127) // 128) * 128 for hw in HWS]   # [10112, 2560, 640, 256]
LB = [0]
for _r in REG[:-1]:
    LB.append(LB[-1] + _r)                # [0, 10112, 12672, 13312]
HEADREG = sum(REG)                         # 13568
NQROWS = 2 * HEADREG                       # 27136
VLB = [0]
for _hw in HWS[:-1]:
    VLB.append(VLB[-1] + _hw)             # [0, 10000, 12500, 13125]
VT_COLS = 13440                            # padded value_T cols

NSLOT = 2500                               # g slots (q = qlo*2500 + g)
G = 48                                     # slots per gather chunk
NCHUNK = 53
NSLOT_PAD = G * NCHUNK                     # 2544
T16_COLS = NSLOT_PAD * 8                   # 20352
QCH = 512                                  # q-column chunk for PE phases
NQCH = (NQ + QCH - 1) // QCH               # 20
# phase-4 column chunks (start, width); widths G-aligned for wrap slicing
PH4CHUNKS = [(0, 672), (672, 624), (1296, 624), (1920, 624)]

_CACHE = {}


def A(t, part, dims, off=0):
    """AP over tile/tensor `t`: part=(start, count, step) in rows/partitions,
    dims=[[step, count], ...] flat-element steps, off = extra element offset."""
    a = t if isinstance(t, bass.AP) else t[:]
    pitch = a.ap[0][0]
    start, cnt, pstep = part
    return bass.AP(tensor=a.tensor, offset=a.offset + start * pitch + off,
                   ap=[[pstep * pitch, cnt]] + [list(d) for d in dims])


def RAW(t, dims, off=0):
    a = t if isinstance(t, bass.AP) else t[:]
    return bass.AP(tensor=a.tensor, offset=a.offset + off,
                   ap=[list(d) for d in dims])


def build_nc(debug=False):
    KPHASE = int(os.environ.get("KPHASE", "9"))
    nc = bacc.Bacc("TRN2", target_bir_lowering=False, debug=False, num_devices=8)

    def din(name, shape, dt=F32):
        return nc.dram_tensor(name, shape, dt, kind="ExternalInput").ap()

    def dout(name, shape, dt=F32):
        return nc.dram_tensor(name, shape, dt, kind="ExternalOutput").ap()

    def dx(name, shape, dt=F32):
        kind = "ExternalOutput" if debug else "Internal"
        return nc.dram_tensor(name, shape, dt, kind=kind).ap()

    vT = din("vT", [256, VT_COLS])
    qT = din("qT", [256, NQ])
    refx = din("refx", [128, NSLOT_PAD])
    refy = din("refy", [128, NSLOT_PAD])
    wvT = din("wvT", [256, 64])
    woffT = din("woffT", [256, 64])
    wattnT = din("wattnT", [256, 32])
    battn = din("battn", [32, 1])
    bvalrep = din("bvalrep", [128, 64])
    woutT = din("woutT", [64, 256])
    ind8 = din("ind8", [128, 8])
    indsum = din("indsum", [32, 2])
    wcol = din("wcol", [128, 1])
    w2col = din("w2col", [128, 1])
    h2col = din("h2col", [128, 1])
    basecol = din("basecol", [128, 1])

    out_T = dout("out_T", [256, NQ])

    quad_dram = nc.dram_tensor("quad_dram", [NQROWS, 128], F32, kind="Internal").ap()
    wrap16 = nc.dram_tensor("wrap16", [16, T16_COLS], I16, kind="Internal").ap()
    stage_dram = nc.dram_tensor("stage_dram", [8, 32 * NSLOT_PAD], F32,
                                kind="Internal").ap()
    off_dram = dx("off_dram", [64, NQ])
    attn_dram = dx("attn_dram", [32, NQ])
    sums_dram = dx("sums_dram", [2, NQ])

    dbg = {}
    if debug:
        for nm, shp, dt in [
            ("d_gx", [128, NSLOT_PAD], F32), ("d_gy", [128, NSLOT_PAD], F32),
            ("d_idx", [128, NSLOT_PAD], I16), ("d_u", [128, NSLOT_PAD, 4], F32),
            ("d_acc", [64, NQ], F32),
            ("d_red1", [128, G * 32], F32), ("d_gt", [128, G * 128], F32),
        ]:
            dbg[nm] = dout(nm, shp, dt)

    def _body():
        with tile.TileContext(nc) as tc:
          with tc.tile_pool(name="consts", bufs=1) as consts, \
               tc.tile_pool(name="persist", bufs=1) as persist:
            # persistent across the gather loop
            u_t = persist.tile([128, NSLOT_PAD, 4], F32)
            idx16 = persist.tile([128, NSLOT_PAD], I16)

            wv_t = consts.tile([128, 2, 64], F32)
            nc.sync.dma_start(out=wv_t[:],
                              in_=A(wvT, (0, 128, 1), [[128 * 64, 2], [1, 64]]))
            bval_t = consts.tile([128, 64], F32)
            nc.sync.dma_start(out=bval_t[:], in_=bvalrep[:])
            battn_t = consts.tile([32, 1], F32)
            nc.sync.dma_start(out=battn_t[:], in_=battn[:])
            indsum_t = consts.tile([32, 2], F32)
            nc.sync.dma_start(out=indsum_t[:], in_=indsum[:])
            ind8_t = consts.tile([128, 8], F32)
            nc.sync.dma_start(out=ind8_t[:], in_=ind8[:])
            woff_t = consts.tile([128, 2, 64], F32)
            nc.sync.dma_start(out=woff_t[:],
                              in_=A(woffT, (0, 128, 1), [[128 * 64, 2], [1, 64]]))
            wattn_t = consts.tile([128, 2, 32], F32)
            nc.sync.dma_start(out=wattn_t[:],
                              in_=A(wattnT, (0, 128, 1), [[128 * 32, 2], [1, 32]]))
            wcol_t = consts.tile([128, 1], F32)
            w2col_t = consts.tile([128, 1], F32)
            h2col_t = consts.tile([128, 1], F32)
            basecol_t = consts.tile([128, 1], F32)
            nc.sync.dma_start(out=wcol_t[:], in_=wcol[:])
            nc.sync.dma_start(out=w2col_t[:], in_=w2col[:])
            nc.sync.dma_start(out=h2col_t[:], in_=h2col[:])
            nc.sync.dma_start(out=basecol_t[:], in_=basecol[:])
            wout_t = consts.tile([64, 256], F32)
            nc.sync.dma_start(out=wout_t[:], in_=woutT[:])
            two_col = consts.tile([128, 1], F32)
            nc.vector.memset(two_col[:], 2.0)

            # ---------------- phase 3: off / attn / softmax -> DRAM -----------
            with tc.tile_pool(name="ph3", bufs=3) as ph3, \
                 tc.tile_pool(name="qt", bufs=3) as qt_pool, \
                 tc.tile_pool(name="p3ps", bufs=2, space="PSUM") as p3ps:
                for qc in range(NQCH):
                    q0 = qc * QCH
                    n = min(QCH, NQ - q0)
                    qt = qt_pool.tile([128, 2, QCH], F32)
                    nc.sync.dma_start(
                        out=qt[:, :, :n],
                        in_=A(qT, (0, 128, 1), [[128 * NQ, 2], [1, n]], off=q0))
                    pso = p3ps.tile([64, QCH], F32)
                    psa = p3ps.tile([32, QCH], F32)
                    for k in range(2):
                        nc.tensor.matmul(pso[:, :n], woff_t[:, k, :], qt[:, k, :n],
                                         start=(k == 0), stop=(k == 1))
                    for k in range(2):
                        nc.tensor.matmul(psa[:, :n], wattn_t[:, k, :], qt[:, k, :n],
                                         start=(k == 0), stop=(k == 1))
                    osb = ph3.tile([64, QCH], F32, tag="osb")
                    asb = ph3.tile([32, QCH], F32, tag="asb")
                    ssb = ph3.tile([2, QCH], F32, tag="ssb")
                    nc.scalar.activation(osb[:, :n], pso[:, :n], ACTF.Copy)
                    nc.scalar.activation(asb[:, :n], psa[:, :n], ACTF.Exp,
                                         bias=battn_t[:])
                    pss = p3ps.tile([2, QCH], F32)
                    nc.tensor.matmul(pss[:, :n], indsum_t[:], asb[:, :n],
                                     start=True, stop=True)
                    nc.scalar.activation(ssb[:, :n], pss[:, :n], ACTF.Copy)
                    nc.sync.dma_start(out=off_dram[:, q0:q0 + n], in_=osb[:, :n])
                    nc.sync.dma_start(out=attn_dram[:, q0:q0 + n], in_=asb[:, :n])
                    nc.sync.dma_start(out=sums_dram[:, q0:q0 + n], in_=ssb[:, :n])

            # ------- phase 3.5: rearrange to 128-partition layout -------------
            # q = qlo*2500 + g  ->  every DMA run is a contiguous 10KB row.
            if KPHASE < 3:
                return
            with tc.tile_pool(name="lay", bufs=1) as lay:
                offx = lay.tile([128, NSLOT_PAD], F32)
                offy = lay.tile([128, NSLOT_PAD], F32)
                attn1 = lay.tile([128, NSLOT_PAD], F32)
                sums1 = lay.tile([128, NSLOT_PAD], F32)
                nc.vector.memset(offx[:], 0.0)
                nc.vector.memset(offy[:], 0.0)
                nc.vector.memset(attn1[:], 0.0)
                nc.vector.memset(sums1[:], 1.0)
                for qlo in range(4):
                    nc.sync.dma_start(
                        out=A(offx, (qlo * 32, 32, 1), [[1, NSLOT]]),
                        in_=A(off_dram, (0, 32, 2), [[1, NSLOT]], off=qlo * NSLOT))
                    nc.sync.dma_start(
                        out=A(offy, (qlo * 32, 32, 1), [[1, NSLOT]]),
                        in_=A(off_dram, (1, 32, 2), [[1, NSLOT]], off=qlo * NSLOT))
                    nc.sync.dma_start(
                        out=A(attn1, (qlo * 32, 32, 1), [[1, NSLOT]]),
                        in_=A(attn_dram, (0, 32, 1), [[1, NSLOT]], off=qlo * NSLOT))
                    for h in range(2):
                        nc.sync.dma_start(
                            out=A(sums1, (qlo * 32 + h * 16, 16, 1), [[1, NSLOT]]),
                            in_=RAW(sums_dram, [[0, 16], [1, NSLOT]],
                                    off=h * NQ + qlo * NSLOT))

                # -------- phase 4: weights + indices (column-chunked) ---------
                with tc.tile_pool(name="p4", bufs=2) as p4:
                    for ci, (c0, CW) in enumerate(PH4CHUNKS):
                        SH = [128, CW]
                        sl = slice(c0, c0 + CW)

                        rfx = p4.tile(SH, F32, tag="rfx")
                        rfy = p4.tile(SH, F32, tag="rfy")
                        nc.sync.dma_start(out=rfx[:], in_=refx[:, sl])
                        nc.sync.dma_start(out=rfy[:], in_=refy[:, sl])
                        gxt = p4.tile(SH, F32, tag="gx")
                        gyt = p4.tile(SH, F32, tag="gy")
                        nc.vector.tensor_tensor(gxt[:], offx[:, sl], rfx[:], ALU.add)
                        nc.vector.tensor_tensor(gyt[:], offy[:, sl], rfy[:], ALU.add)
                        rcp = p4.tile(SH, F32, tag="rcp")
                        nc.vector.reciprocal(rcp[:], sums1[:, sl])

                        tt = p4.tile(SH, F32, tag="tt")
                        t2 = p4.tile(SH, F32, tag="t2")
                        ti = p4.tile(SH, I32, tag="ti")
                        x0c = p4.tile(SH, F32, tag="x0c")
                        y0c = p4.tile(SH, F32, tag="y0c")
                        # floor(g): c = int(g - 0.5) (round OR trunc semantics both
                        # give c in {floor-1, floor} for g>0); fix up with
                        # +1 if g - c >= 1. Negative g cases are absorbed by the
                        # clamp + hat-weight formulation.
                        for g_t, cl_t, out_t in ((gxt, w2col_t, x0c),
                                                 (gyt, h2col_t, y0c)):
                            nc.vector.tensor_scalar(tt[:], g_t[:], 0.5, None,
                                                    ALU.subtract)
                            nc.vector.tensor_copy(ti[:], tt[:])
                            nc.vector.tensor_copy(tt[:], ti[:])
                            nc.vector.tensor_tensor(t2[:], g_t[:], tt[:], ALU.subtract)
                            nc.vector.tensor_scalar(t2[:], t2[:], 1.0, None, ALU.is_ge)
                            nc.vector.tensor_tensor(tt[:], tt[:], t2[:], ALU.add)
                            nc.vector.tensor_scalar(out_t[:], tt[:], 0.0, cl_t[:],
                                                    ALU.max, ALU.min)

                        # hat weights: hat(d) = min(relu(1-d), relu(1+d)),
                        # shifted hat: hat(d-1) = min(relu(d), relu(2-d))
                        dxt = p4.tile(SH, F32, tag="dxt")
                        dyt = p4.tile(SH, F32, tag="dyt")
                        wx0 = p4.tile(SH, F32, tag="wx0")
                        wx1 = p4.tile(SH, F32, tag="wx1")
                        wy0 = p4.tile(SH, F32, tag="wy0")
                        wy1 = p4.tile(SH, F32, tag="wy1")
                        ra = p4.tile(SH, F32, tag="ra")
                        nc.vector.tensor_tensor(dxt[:], gxt[:], x0c[:], ALU.subtract)
                        nc.scalar.activation(ra[:], dxt[:], ACTF.Relu,
                                             bias=1.0, scale=-1.0)
                        nc.scalar.activation(tt[:], dxt[:], ACTF.Relu,
                                             bias=1.0, scale=1.0)
                        nc.vector.tensor_tensor(wx0[:], ra[:], tt[:], ALU.min)
                        nc.scalar.activation(ra[:], dxt[:], ACTF.Relu,
                                             bias=0.0, scale=1.0)
                        nc.scalar.activation(tt[:], dxt[:], ACTF.Relu,
                                             bias=two_col[:], scale=-1.0)
                        nc.vector.tensor_tensor(wx1[:], ra[:], tt[:], ALU.min)
                        nc.vector.tensor_tensor(dyt[:], gyt[:], y0c[:], ALU.subtract)
                        nc.scalar.activation(ra[:], dyt[:], ACTF.Relu,
                                             bias=1.0, scale=-1.0)
                        nc.scalar.activation(tt[:], dyt[:], ACTF.Relu,
                                             bias=1.0, scale=1.0)
                        nc.vector.tensor_tensor(wy0[:], ra[:], tt[:], ALU.min)
                        nc.scalar.activation(ra[:], dyt[:], ACTF.Relu,
                                             bias=0.0, scale=1.0)
                        nc.scalar.activation(tt[:], dyt[:], ACTF.Relu,
                                             bias=two_col[:], scale=-1.0)
                        nc.vector.tensor_tensor(wy1[:], ra[:], tt[:], ALU.min)

                        ubt = p4.tile(SH, F32, tag="ubt")
                        nc.vector.tensor_tensor(ubt[:], attn1[:, sl], rcp[:], ALU.mult)
                        uy0 = p4.tile(SH, F32, tag="uy0")
                        uy1 = p4.tile(SH, F32, tag="uy1")
                        nc.vector.tensor_tensor(uy0[:], ubt[:], wy0[:], ALU.mult)
                        nc.vector.tensor_tensor(uy1[:], ubt[:], wy1[:], ALU.mult)
                        for c, (uy, wx) in enumerate([(uy0, wx0), (uy0, wx1),
                                                      (uy1, wx0), (uy1, wx1)]):
                            nc.vector.tensor_tensor(
                                A(u_t, (0, 128, 1), [[4, CW]], off=c0 * 4 + c),
                                uy[:], wx[:], ALU.mult)

                        nc.vector.tensor_scalar(tt[:], y0c[:], wcol_t[:],
                                                basecol_t[:], ALU.mult, ALU.add)
                        nc.vector.tensor_tensor(tt[:], tt[:], x0c[:], ALU.add)
                        nc.vector.tensor_copy(idx16[:, sl], tt[:])

                        # wrap this chunk's idx cols into the 16-partition
                        # gather stream layout; alternate the two HWDGE queues
                        # (SP / ACT) so the stride-8 2B writes overlap.
                        if KPHASE >= 4:
                            for phi in range(8):
                                qlo, sh = phi // 2, phi % 2
                                eng = (nc.sync, nc.scalar)[phi % 2]
                                eng.dma_start(
                                    out=A(wrap16, (0, 16, 1), [[8, CW]],
                                          off=phi + c0 * 8),
                                    in_=A(idx16, (qlo * 32 + sh * 16, 16, 1),
                                          [[1, CW]], off=c0))

                        if debug:
                            nc.sync.dma_start(out=dbg["d_gx"][:, sl], in_=gxt[:])
                            nc.sync.dma_start(out=dbg["d_gy"][:, sl], in_=gyt[:])

            if debug:
                nc.sync.dma_start(out=dbg["d_idx"][:], in_=idx16[:])
                nc.sync.dma_start(out=dbg["d_u"][:], in_=u_t[:])

            # ---------------- phase 1: value proj -> quad rows ----------------
            GRP = 12  # pixel-chunks per DMA group
            with tc.tile_pool(name="vt", bufs=1) as vt_pool, \
                 tc.tile_pool(name="gb", bufs=2) as gb_pool, \
                 tc.tile_pool(name="qps", bufs=4, space="PSUM") as qps_pool:
                vt = vt_pool.tile([128, 2, VT_COLS], F32)
                nc.sync.dma_start(
                    out=vt[:],
                    in_=A(vT, (0, 128, 1), [[128 * VT_COLS, 2], [1, VT_COLS]]))
                for lvl in range(LEVELS):
                    Hl, Wl = SHAPES[lvl]
                    nch = (HWS[lvl] + 127) // 128
                    for g0 in range(0, nch, GRP):
                        ng = min(GRP, nch - g0)
                        gb = gb_pool.tile([128, GRP, 256], F32)
                        for j in range(ng):
                            pix = (g0 + j) * 128
                            ps = qps_pool.tile([128, 256], F32)
                            for si, shift in enumerate([0, 1, Wl, Wl + 1]):
                                col = VLB[lvl] + pix + shift
                                for k in range(2):
                                    nc.tensor.matmul(
                                        ps[:, si * 64:(si + 1) * 64],
                                        vt[:, k, col:col + 128],
                                        wv_t[:, k, :],
                                        start=(k == 0), stop=(k == 1))
                            # psum cols (shift, h, d) -> gb cols (h, shift, d) + bias
                            nc.vector.tensor_tensor(
                                A(gb, (0, 128, 1), [[128, 2], [32, 4], [1, 32]],
                                  off=j * 256),
                                A(ps, (0, 128, 1), [[32, 2], [64, 4], [1, 32]]),
                                A(bval_t, (0, 128, 1), [[32, 2], [0, 4], [1, 32]]),
                                ALU.add)
                        for h in range(2):
                            row0 = h * HEADREG + LB[lvl] + g0 * 128
                            nc.sync.dma_start(
                                out=A(quad_dram, (row0, 128, 1),
                                      [[128 * 128, ng], [1, 128]]),
                                in_=A(gb, (0, 128, 1), [[256, ng], [1, 128]],
                                      off=h * 128))

            # ---------------- phase 6: gather + weighted reduce ---------------
            if KPHASE < 5:
                return
            with tc.tile_pool(name="gl", bufs=1) as gl, \
                 tc.tile_pool(name="gt", bufs=2) as gt_pool, \
                 tc.tile_pool(name="prod", bufs=1) as prod_pool, \
                 tc.tile_pool(name="red", bufs=2) as red_pool, \
                 tc.tile_pool(name="stg", bufs=2) as stg_pool, \
                 tc.tile_pool(name="gps", bufs=4, space="PSUM") as gps_pool:
                t16 = gl.tile([128, T16_COLS], I16)
                nc.sync.dma_start(
                    out=t16[:],
                    in_=RAW(wrap16, [[0, 8], [T16_COLS, 16], [1, T16_COLS]]))

                for c in range(NCHUNK):
                    gt = gt_pool.tile([128, G, 128], F32)
                    nc.gpsimd.dma_gather(
                        gt[:], quad_dram[:], t16[:, c * (G * 8):(c + 1) * (G * 8)],
                        G * 128, G * 128, 128, single_packet=False)
                    prod = prod_pool.tile([128, G * 128], F32)
                    nc.vector.tensor_tensor(
                        prod[:], gt[:],
                        A(u_t, (0, 128, 1), [[4, G], [1, 4], [0, 32]], off=c * G * 4),
                        ALU.mult)
                    red1 = red_pool.tile([128, G * 32], F32)
                    nc.vector.tensor_reduce(
                        red1[:],
                        A(prod, (0, 128, 1), [[128, G], [1, 32], [32, 4]]),
                        op=ALU.add, axis=mybir.AxisListType.X)
                    # stage in (d, g) order: stg[m, d*G + g]
                    stg = stg_pool.tile([8, 32 * G], F32)
                    for j in range(3):
                        ps = gps_pool.tile([8, QCH], F32)
                        nc.tensor.matmul(ps[:], ind8_t[:],
                                         red1[:, j * QCH:(j + 1) * QCH],
                                         start=True, stop=True)
                        nc.scalar.activation(
                            A(stg, (0, 8, 1), [[1, 16], [G, 32]], off=j * 16),
                            A(ps, (0, 8, 1), [[32, 16], [1, 32]]),
                            ACTF.Copy)
                    nc.sync.dma_start(
                        out=A(stage_dram, (0, 8, 1), [[NSLOT_PAD, 32], [1, G]],
                              off=c * G),
                        in_=A(stg, (0, 8, 1), [[G, 32], [1, G]]))
                    if debug and c == 0:
                        nc.sync.dma_start(out=dbg["d_gt"][:], in_=gt[:])
                        nc.sync.dma_start(out=dbg["d_red1"][:], in_=red1[:])

            # ---------------- phase 7: out projection -------------------------
            if KPHASE < 6:
                return
            with tc.tile_pool(name="ph7", bufs=1) as ph7, \
                 tc.tile_pool(name="p7ps", bufs=4, space="PSUM") as p7ps:
                acc_T = ph7.tile([64, NQ], F32)
                for qlo in range(4):
                    for h in range(2):
                        m = qlo * 2 + h
                        nc.sync.dma_start(
                            out=A(acc_T, (h * 32, 32, 1), [[1, NSLOT]],
                                  off=qlo * NSLOT),
                            in_=A(stage_dram, (m, 1, 1),
                                  [[NSLOT_PAD, 32], [1, NSLOT]]))
                if debug:
                    nc.sync.dma_start(out=dbg["d_acc"][:], in_=acc_T[:])
                osb = ph7.tile([128, 2, NQ], F32)
                for qc in range(NQCH):
                    q0 = qc * QCH
                    n = min(QCH, NQ - q0)
                    for mh in range(2):
                        ps = p7ps.tile([128, QCH], F32)
                        nc.tensor.matmul(ps[:, :n], wout_t[:, mh * 128:(mh + 1) * 128],
                                         acc_T[:, q0:q0 + n], start=True, stop=True)
                        nc.scalar.activation(osb[:, mh, q0:q0 + n], ps[:, :n], ACTF.Copy)
                nc.sync.dma_start(
                    out=A(out_T, (0, 128, 1), [[128 * NQ, 2], [1, NQ]]),
                    in_=osb[:])

    _body()
    nc.compile()
    return nc


def _prep_core_inputs(b, hg, query, value, reference_points,
                      W_off, b_off, W_attn, b_attn, W_val, b_val, W_out):
    """Host-side per-core input dict (all f32 numpy)."""
    f = np.float32
    vT = np.zeros((256, VT_COLS), f)
    vT[:, :NV] = value[b].T
    qT = np.ascontiguousarray(query[b].T.astype(f))

    s_arr = np.arange(32)
    h_loc = s_arr // 16
    l_arr = (s_arr // 4) % 4
    Wl = np.array([SHAPES[l][1] for l in l_arr], f)
    Hl = np.array([SHAPES[l][0] for l in l_arr], f)
    base = np.array([h_loc[s] * HEADREG + LB[l_arr[s]] for s in range(32)], f)

    boff = b_off[hg * 64:(hg + 1) * 64].astype(f)  # rows (s, xy)
    refx = np.zeros((128, NSLOT_PAD), f)
    refy = np.zeros((128, NSLOT_PAD), f)
    ref = np.asarray(reference_points)[b, :, 0, :].astype(f)  # [NQ, 2] (x, y)
    for qlo in range(4):
        rx = ref[qlo * NSLOT + np.arange(NSLOT), 0]
        ry = ref[qlo * NSLOT + np.arange(NSLOT), 1]
        for s in range(32):
            refx[qlo * 32 + s, :NSLOT] = rx * Wl[s] - 0.5 + boff[s * 2 + 0]
            refy[qlo * 32 + s, :NSLOT] = ry * Hl[s] - 0.5 + boff[s * 2 + 1]

    wvT = np.ascontiguousarray(W_val[hg * 64:(hg + 1) * 64, :].T.astype(f))
    woffT = np.ascontiguousarray(W_off[hg * 64:(hg + 1) * 64, :].T.astype(f))
    wattnT = np.ascontiguousarray(W_attn[hg * 32:(hg + 1) * 32, :].T.astype(f))
    battn = b_attn[hg * 32:(hg + 1) * 32].astype(f).reshape(32, 1)
    bvalrep = np.tile(b_val[hg * 64:(hg + 1) * 64].astype(f)[None, :], (128, 1))
    woutT = np.ascontiguousarray(W_out[:, hg * 64:(hg + 1) * 64].T.astype(f))

    p_arr = np.arange(128)
    ind8 = np.zeros((128, 8), f)
    ind8[p_arr, (p_arr // 32) * 2 + (p_arr % 32) // 16] = 1.0
    indsum = np.zeros((32, 2), f)
    indsum[np.arange(32), np.arange(32) // 16] = 1.0

    pcol = lambda v: np.ascontiguousarray(np.tile(v.astype(f), 4).reshape(128, 1))
    return {
        "vT": vT, "qT": qT, "refx": refx, "refy": refy,
        "wvT": wvT, "woffT": woffT, "wattnT": wattnT, "battn": battn,
        "bvalrep": bvalrep, "woutT": woutT, "ind8": ind8, "indsum": indsum,
        "wcol": pcol(Wl), "w2col": pcol(Wl - 2), "h2col": pcol(Hl - 2),
        "basecol": pcol(base),
    }


def run_cores(inputs, debug=False, trace=False):
    key = ("nc", debug)
    if key not in _CACHE:
        _CACHE[key] = build_nc(debug=debug)
    nc = _CACHE[key]
    in_maps = [_prep_core_inputs(c // 4, c % 4, **inputs) for c in range(8)]
    res = bass_utils.run_bass_kernel_spmd(nc, in_maps, core_ids=list(range(8)),
                                          trace=trace)
    return res


def kernel(query, value, reference_points, spatial_shapes,
           W_off, b_off, W_attn, b_attn, W_val, b_val, W_out, b_out,
           _debug=False, _trace=False):
    inputs = dict(query=np.asarray(query), value=np.asarray(value),
                  reference_points=np.asarray(reference_points),
                  W_off=np.asarray(W_off), b_off=np.asarray(b_off),
                  W_attn=np.asarray(W_attn), b_attn=np.asarray(b_attn),
                  W_val=np.asarray(W_val), b_val=np.asarray(b_val),
                  W_out=np.asarray(W_out))
    res = run_cores(inputs, debug=_debug, trace=_trace)
    out = np.zeros((BS, NQ, 256), np.float32)
    for b in range(BS):
        acc = np.zeros((256, NQ), np.float32)
        for hg in range(4):
            acc += res.results[b * 4 + hg]["out_T"]
        out[b] = acc.T + np.asarray(b_out)[None, :].astype(np.float32)
    kernel._last_res = res
    return out


# revision 10
# speedup vs baseline: 2.0025x; 1.5227x over previous
"""Multi-scale deformable attention on 8 Trainium2 NeuronCores (Bass/Tile).

Sharding: core c = (batch b = c//4, head-pair hg = c%4) — each core handles 2
of the 8 heads for one batch element, all 10000 queries. Host sums the 4
partial output projections per batch and adds b_out.

Per-core pipeline (ordered so the SWDGE gather desc-gen — the serial floor —
starts as early as possible):
  1. PE: offset / attention projections from query_T; softmax via ACT exp +
     PE indicator-sum + DVE reciprocal.  -> off/attn/sums in DRAM.
  2. Relayout to 128-partition (qblock, s) layout: q = qlo*2500 + g so every
     DMA run is contiguous (no 4-byte packets).
  3. DVE: sampling grid -> clamped corner + hat weights, fused attn*recip
     weights u, int16 row idx; wrap idx into the 16-row gather stream layout
     (stride-8 interleave DMAs split across the SP + ACT HWDGE queues).
  4. PE: value projection with 4 pixel-shifts (0, 1, W, W+1) -> "quad rows"
     [v(y0,x0), v(y0,x0+1), v(y1,x0), v(y1,x1)] per head per pixel, stored to
     DRAM as [NQROWS, 128] f32 (512B rows).
  5. Loop: SWDGE dma_gather of 512B quad rows (partition = (qlo,s) sample
     slot), DVE broadcast-weight multiply + corner reduce, PE indicator
     matmul reduces 16 (level,point) partitions -> per-(q,head) accumulators,
     staged to DRAM in (d, g) order so phase-7 loads are contiguous.
  6. PE: output projection W_out slice; host sums head-pair partials.
"""

import os

import numpy as np

import concourse.bass as bass
import concourse.bacc as bacc
import concourse.tile as tile
from concourse import mybir
from concourse import bass_utils

F32 = mybir.dt.float32
I32 = mybir.dt.int32
I16 = mybir.dt.int16
ALU = mybir.AluOpType
ACTF = mybir.ActivationFunctionType

# ---- problem constants (hardcoded; must match reference) ----
EMBED = 256
HEADS = 8
LEVELS = 4
POINTS = 4
D = 32
SHAPES = [(100, 100), (50, 50), (25, 25), (13, 13)]
HWS = [h * w for h, w in SHAPES]          # [10000, 2500, 625, 169]
NV = sum(HWS)                              # 13294
BS = 2
NQ = 10000

# quad-row (padded) level layout, per head
REG = [((hw + 127) // 128) * 128 for hw in HWS]   # [10112, 2560, 640, 256]
LB = [0]
for _r in REG[:-1]:
    LB.append(LB[-1] + _r)                # [0, 10112, 12672, 13312]
HEADREG = sum(REG)                         # 13568
NQROWS = 2 * HEADREG                       # 27136
VLB = [0]
for _hw in HWS[:-1]:
    VLB.append(VLB[-1] + _hw)             # [0, 10000, 12500, 13125]
VT_COLS = 13440                            # padded value_T cols

NSLOT = 2500                               # g slots (q = qlo*2500 + g)
G = 48                                     # slots per gather chunk
NCHUNK = 53
NSLOT_PAD = G * NCHUNK                     # 2544
T16_COLS = NSLOT_PAD * 8                   # 20352
QCH = 512                                  # q-column chunk for PE phases
NQCH = (NQ + QCH - 1) // QCH               # 20
# phase-4 column chunks (start, width); widths G-aligned for wrap slicing
PH4CHUNKS = [(0, 672), (672, 624), (1296, 624), (1920, 624)]

_CACHE = {}


def A(t, part, dims, off=0):
    """AP over tile/tensor `t`: part=(start, count, step) in rows/partitions,
    dims=[[step, count], ...] flat-element steps, off = extra element offset."""
    a = t if isinstance(t, bass.AP) else t[:]
    pitch = a.ap[0][0]
    start, cnt, pstep = part
    return bass.AP(tensor=a.tensor, offset=a.offset + start * pitch + off,
                   ap=[[pstep * pitch, cnt]] + [list(d) for d in dims])


def RAW(t, dims, off=0):
    a = t if isinstance(t, bass.AP) else t[:]
    return bass.AP(tensor=a.tensor, offset=a.offset + off,
                   ap=[list(d) for d in dims])


def build_nc(debug=False):
    KPHASE = int(os.environ.get("KPHASE", "9"))
    nc = bacc.Bacc("TRN2", target_bir_lowering=False, debug=False, num_devices=8)

    def din(name, shape, dt=F32):
        return nc.dram_tensor(name, shape, dt, kind="ExternalInput").ap()

    def dout(name, shape, dt=F32):
        return nc.dram_tensor(name, shape, dt, kind="ExternalOutput").ap()

    def dx(name, shape, dt=F32):
        kind = "ExternalOutput" if debug else "Internal"
        return nc.dram_tensor(name, shape, dt, kind=kind).ap()

    vT = din("vT", [256, VT_COLS])
    qT = din("qT", [256, NQ])
    refx = din("refx", [128, NSLOT_PAD])
    refy = din("refy", [128, NSLOT_PAD])
    wvT = din("wvT", [256, 64])
    woffT = din("woffT", [256, 64])
    wattnT = din("wattnT", [256, 32])
    battn = din("battn", [32, 1])
    bvalrep = din("bvalrep", [128, 64])
    woutT = din("woutT", [64, 256])
    ind8 = din("ind8", [128, 8])
    indsum = din("indsum", [32, 2])
    wcol = din("wcol", [128, 1])
    w2col = din("w2col", [128, 1])
    h2col = din("h2col", [128, 1])
    basecol = din("basecol", [128, 1])

    out_T = dout("out_T", [256, NQ])

    quad_dram = nc.dram_tensor("quad_dram", [NQROWS, 128], F32, kind="Internal").ap()
    idx_dram = nc.dram_tensor("idx_dram", [128, NSLOT_PAD], I16, kind="Internal").ap()
    stage_dram = nc.dram_tensor("stage_dram", [8, 32 * NSLOT_PAD], F32,
                                kind="Internal").ap()
    off_dram = dx("off_dram", [64, NQ])
    attn_dram = dx("attn_dram", [32, NQ])
    sums_dram = dx("sums_dram", [2, NQ])

    dbg = {}
    if debug:
        for nm, shp, dt in [
            ("d_gx", [128, NSLOT_PAD], F32), ("d_gy", [128, NSLOT_PAD], F32),
            ("d_idx", [128, NSLOT_PAD], I16), ("d_u", [128, NSLOT_PAD, 4], F32),
            ("d_acc", [64, NQ], F32),
            ("d_red1", [128, G * 32], F32), ("d_gt", [128, G * 128], F32),
        ]:
            dbg[nm] = dout(nm, shp, dt)

    def _body():
        with tile.TileContext(nc) as tc:
          with tc.tile_pool(name="consts", bufs=1) as consts, \
               tc.tile_pool(name="persist", bufs=1) as persist:
            # persistent across the gather loop
            u_t = persist.tile([128, NSLOT_PAD, 4], F32)
            idx16 = persist.tile([128, NSLOT_PAD], I16)

            wv_t = consts.tile([128, 2, 64], F32)
            nc.sync.dma_start(out=wv_t[:],
                              in_=A(wvT, (0, 128, 1), [[128 * 64, 2], [1, 64]]))
            bval_t = consts.tile([128, 64], F32)
            nc.sync.dma_start(out=bval_t[:], in_=bvalrep[:])
            battn_t = consts.tile([32, 1], F32)
            nc.sync.dma_start(out=battn_t[:], in_=battn[:])
            indsum_t = consts.tile([32, 2], F32)
            nc.sync.dma_start(out=indsum_t[:], in_=indsum[:])
            ind8_t = consts.tile([128, 8], F32)
            nc.sync.dma_start(out=ind8_t[:], in_=ind8[:])
            woff_t = consts.tile([128, 2, 64], F32)
            nc.sync.dma_start(out=woff_t[:],
                              in_=A(woffT, (0, 128, 1), [[128 * 64, 2], [1, 64]]))
            wattn_t = consts.tile([128, 2, 32], F32)
            nc.sync.dma_start(out=wattn_t[:],
                              in_=A(wattnT, (0, 128, 1), [[128 * 32, 2], [1, 32]]))
            wcol_t = consts.tile([128, 1], F32)
            w2col_t = consts.tile([128, 1], F32)
            h2col_t = consts.tile([128, 1], F32)
            basecol_t = consts.tile([128, 1], F32)
            nc.sync.dma_start(out=wcol_t[:], in_=wcol[:])
            nc.sync.dma_start(out=w2col_t[:], in_=w2col[:])
            nc.sync.dma_start(out=h2col_t[:], in_=h2col[:])
            nc.sync.dma_start(out=basecol_t[:], in_=basecol[:])
            wout_t = consts.tile([64, 256], F32)
            nc.sync.dma_start(out=wout_t[:], in_=woutT[:])
            two_col = consts.tile([128, 1], F32)
            nc.vector.memset(two_col[:], 2.0)

            # ---------------- phase 3: off / attn / softmax -> DRAM -----------
            with tc.tile_pool(name="ph3", bufs=3) as ph3, \
                 tc.tile_pool(name="qt", bufs=3) as qt_pool, \
                 tc.tile_pool(name="p3ps", bufs=2, space="PSUM") as p3ps:
                for qc in range(NQCH):
                    q0 = qc * QCH
                    n = min(QCH, NQ - q0)
                    qt = qt_pool.tile([128, 2, QCH], F32)
                    nc.sync.dma_start(
                        out=qt[:, :, :n],
                        in_=A(qT, (0, 128, 1), [[128 * NQ, 2], [1, n]], off=q0))
                    pso = p3ps.tile([64, QCH], F32)
                    psa = p3ps.tile([32, QCH], F32)
                    for k in range(2):
                        nc.tensor.matmul(pso[:, :n], woff_t[:, k, :], qt[:, k, :n],
                                         start=(k == 0), stop=(k == 1))
                    for k in range(2):
                        nc.tensor.matmul(psa[:, :n], wattn_t[:, k, :], qt[:, k, :n],
                                         start=(k == 0), stop=(k == 1))
                    osb = ph3.tile([64, QCH], F32, tag="osb")
                    asb = ph3.tile([32, QCH], F32, tag="asb")
                    ssb = ph3.tile([2, QCH], F32, tag="ssb")
                    nc.scalar.activation(osb[:, :n], pso[:, :n], ACTF.Copy)
                    nc.scalar.activation(asb[:, :n], psa[:, :n], ACTF.Exp,
                                         bias=battn_t[:])
                    pss = p3ps.tile([2, QCH], F32)
                    nc.tensor.matmul(pss[:, :n], indsum_t[:], asb[:, :n],
                                     start=True, stop=True)
                    nc.scalar.activation(ssb[:, :n], pss[:, :n], ACTF.Copy)
                    nc.sync.dma_start(out=off_dram[:, q0:q0 + n], in_=osb[:, :n])
                    nc.sync.dma_start(out=attn_dram[:, q0:q0 + n], in_=asb[:, :n])
                    nc.sync.dma_start(out=sums_dram[:, q0:q0 + n], in_=ssb[:, :n])

            # ------- phase 3.5: rearrange to 128-partition layout -------------
            # q = qlo*2500 + g  ->  every DMA run is a contiguous 10KB row.
            if KPHASE < 3:
                return
            with tc.tile_pool(name="lay", bufs=1) as lay:
                offx = lay.tile([128, NSLOT_PAD], F32)
                offy = lay.tile([128, NSLOT_PAD], F32)
                attn1 = lay.tile([128, NSLOT_PAD], F32)
                sums1 = lay.tile([128, NSLOT_PAD], F32)
                nc.vector.memset(offx[:], 0.0)
                nc.vector.memset(offy[:], 0.0)
                nc.vector.memset(attn1[:], 0.0)
                nc.vector.memset(sums1[:], 1.0)
                for qlo in range(4):
                    nc.sync.dma_start(
                        out=A(offx, (qlo * 32, 32, 1), [[1, NSLOT]]),
                        in_=A(off_dram, (0, 32, 2), [[1, NSLOT]], off=qlo * NSLOT))
                    nc.sync.dma_start(
                        out=A(offy, (qlo * 32, 32, 1), [[1, NSLOT]]),
                        in_=A(off_dram, (1, 32, 2), [[1, NSLOT]], off=qlo * NSLOT))
                    nc.sync.dma_start(
                        out=A(attn1, (qlo * 32, 32, 1), [[1, NSLOT]]),
                        in_=A(attn_dram, (0, 32, 1), [[1, NSLOT]], off=qlo * NSLOT))
                    for h in range(2):
                        nc.sync.dma_start(
                            out=A(sums1, (qlo * 32 + h * 16, 16, 1), [[1, NSLOT]]),
                            in_=RAW(sums_dram, [[0, 16], [1, NSLOT]],
                                    off=h * NQ + qlo * NSLOT))

                # -------- phase 4: weights + indices (column-chunked) ---------
                with tc.tile_pool(name="p4", bufs=2) as p4:
                    for ci, (c0, CW) in enumerate(PH4CHUNKS):
                        SH = [128, CW]
                        sl = slice(c0, c0 + CW)

                        rfx = p4.tile(SH, F32, tag="rfx")
                        rfy = p4.tile(SH, F32, tag="rfy")
                        nc.sync.dma_start(out=rfx[:], in_=refx[:, sl])
                        nc.sync.dma_start(out=rfy[:], in_=refy[:, sl])
                        gxt = p4.tile(SH, F32, tag="gx")
                        gyt = p4.tile(SH, F32, tag="gy")
                        nc.vector.tensor_tensor(gxt[:], offx[:, sl], rfx[:], ALU.add)
                        nc.vector.tensor_tensor(gyt[:], offy[:, sl], rfy[:], ALU.add)
                        rcp = p4.tile(SH, F32, tag="rcp")
                        nc.vector.reciprocal(rcp[:], sums1[:, sl])

                        tt = p4.tile(SH, F32, tag="tt")
                        t2 = p4.tile(SH, F32, tag="t2")
                        ti = p4.tile(SH, I32, tag="ti")
                        x0c = p4.tile(SH, F32, tag="x0c")
                        y0c = p4.tile(SH, F32, tag="y0c")
                        # floor(g): c = int(g - 0.5) (round OR trunc semantics both
                        # give c in {floor-1, floor} for g>0); fix up with
                        # +1 if g - c >= 1. Negative g cases are absorbed by the
                        # clamp + hat-weight formulation.
                        for g_t, cl_t, out_t in ((gxt, w2col_t, x0c),
                                                 (gyt, h2col_t, y0c)):
                            nc.vector.tensor_scalar(tt[:], g_t[:], 0.5, None,
                                                    ALU.subtract)
                            nc.vector.tensor_copy(ti[:], tt[:])
                            nc.vector.tensor_copy(tt[:], ti[:])
                            nc.vector.tensor_tensor(t2[:], g_t[:], tt[:], ALU.subtract)
                            nc.vector.tensor_scalar(t2[:], t2[:], 1.0, None, ALU.is_ge)
                            nc.vector.tensor_tensor(tt[:], tt[:], t2[:], ALU.add)
                            nc.vector.tensor_scalar(out_t[:], tt[:], 0.0, cl_t[:],
                                                    ALU.max, ALU.min)

                        # hat weights: hat(d) = min(relu(1-d), relu(1+d)),
                        # shifted hat: hat(d-1) = min(relu(d), relu(2-d))
                        dxt = p4.tile(SH, F32, tag="dxt")
                        dyt = p4.tile(SH, F32, tag="dyt")
                        wx0 = p4.tile(SH, F32, tag="wx0")
                        wx1 = p4.tile(SH, F32, tag="wx1")
                        wy0 = p4.tile(SH, F32, tag="wy0")
                        wy1 = p4.tile(SH, F32, tag="wy1")
                        ra = p4.tile(SH, F32, tag="ra")
                        nc.vector.tensor_tensor(dxt[:], gxt[:], x0c[:], ALU.subtract)
                        nc.scalar.activation(ra[:], dxt[:], ACTF.Relu,
                                             bias=1.0, scale=-1.0)
                        nc.scalar.activation(tt[:], dxt[:], ACTF.Relu,
                                             bias=1.0, scale=1.0)
                        nc.vector.tensor_tensor(wx0[:], ra[:], tt[:], ALU.min)
                        nc.scalar.activation(ra[:], dxt[:], ACTF.Relu,
                                             bias=0.0, scale=1.0)
                        nc.scalar.activation(tt[:], dxt[:], ACTF.Relu,
                                             bias=two_col[:], scale=-1.0)
                        nc.vector.tensor_tensor(wx1[:], ra[:], tt[:], ALU.min)
                        nc.vector.tensor_tensor(dyt[:], gyt[:], y0c[:], ALU.subtract)
                        nc.scalar.activation(ra[:], dyt[:], ACTF.Relu,
                                             bias=1.0, scale=-1.0)
                        nc.scalar.activation(tt[:], dyt[:], ACTF.Relu,
                                             bias=1.0, scale=1.0)
                        nc.vector.tensor_tensor(wy0[:], ra[:], tt[:], ALU.min)
                        nc.scalar.activation(ra[:], dyt[:], ACTF.Relu,
                                             bias=0.0, scale=1.0)
                        nc.scalar.activation(tt[:], dyt[:], ACTF.Relu,
                                             bias=two_col[:], scale=-1.0)
                        nc.vector.tensor_tensor(wy1[:], ra[:], tt[:], ALU.min)

                        ubt = p4.tile(SH, F32, tag="ubt")
                        nc.vector.tensor_tensor(ubt[:], attn1[:, sl], rcp[:], ALU.mult)
                        uy0 = p4.tile(SH, F32, tag="uy0")
                        uy1 = p4.tile(SH, F32, tag="uy1")
                        nc.vector.tensor_tensor(uy0[:], ubt[:], wy0[:], ALU.mult)
                        nc.vector.tensor_tensor(uy1[:], ubt[:], wy1[:], ALU.mult)
                        for c, (uy, wx) in enumerate([(uy0, wx0), (uy0, wx1),
                                                      (uy1, wx0), (uy1, wx1)]):
                            nc.vector.tensor_tensor(
                                A(u_t, (0, 128, 1), [[4, CW]], off=c0 * 4 + c),
                                uy[:], wx[:], ALU.mult)

                        nc.vector.tensor_scalar(tt[:], y0c[:], wcol_t[:],
                                                basecol_t[:], ALU.mult, ALU.add)
                        nc.vector.tensor_tensor(tt[:], tt[:], x0c[:], ALU.add)
                        nc.vector.tensor_copy(idx16[:, sl], tt[:])
                        if c0 + CW == NSLOT_PAD:
                            # pad slots -> -1 so the gather ucode trims the
                            # trailing descriptors of the last chunk
                            nc.vector.memset(idx16[:, NSLOT:NSLOT_PAD], -1.0)
                        nc.sync.dma_start(out=idx_dram[:, sl], in_=idx16[:, sl])

                        if debug:
                            nc.sync.dma_start(out=dbg["d_gx"][:, sl], in_=gxt[:])
                            nc.sync.dma_start(out=dbg["d_gy"][:, sl], in_=gyt[:])

            if debug:
                nc.sync.dma_start(out=dbg["d_idx"][:], in_=idx16[:])
                nc.sync.dma_start(out=dbg["d_u"][:], in_=u_t[:])

            # ---------------- phase 1: value proj -> quad rows ----------------
            GRP = 12  # pixel-chunks per DMA group
            with tc.tile_pool(name="vt", bufs=1) as vt_pool, \
                 tc.tile_pool(name="gb", bufs=2) as gb_pool, \
                 tc.tile_pool(name="qps", bufs=4, space="PSUM") as qps_pool:
                vt = vt_pool.tile([128, 2, VT_COLS], F32)
                nc.sync.dma_start(
                    out=vt[:],
                    in_=A(vT, (0, 128, 1), [[128 * VT_COLS, 2], [1, VT_COLS]]))
                for lvl in range(LEVELS):
                    Hl, Wl = SHAPES[lvl]
                    nch = (HWS[lvl] + 127) // 128
                    for g0 in range(0, nch, GRP):
                        ng = min(GRP, nch - g0)
                        gb = gb_pool.tile([128, GRP, 256], F32)
                        for j in range(ng):
                            pix = (g0 + j) * 128
                            ps = qps_pool.tile([128, 256], F32)
                            for si, shift in enumerate([0, 1, Wl, Wl + 1]):
                                col = VLB[lvl] + pix + shift
                                for k in range(2):
                                    nc.tensor.matmul(
                                        ps[:, si * 64:(si + 1) * 64],
                                        vt[:, k, col:col + 128],
                                        wv_t[:, k, :],
                                        start=(k == 0), stop=(k == 1))
                            # psum cols (shift, h, d) -> gb cols (h, shift, d) + bias
                            nc.vector.tensor_tensor(
                                A(gb, (0, 128, 1), [[128, 2], [32, 4], [1, 32]],
                                  off=j * 256),
                                A(ps, (0, 128, 1), [[32, 2], [64, 4], [1, 32]]),
                                A(bval_t, (0, 128, 1), [[32, 2], [0, 4], [1, 32]]),
                                ALU.add)
                        for h in range(2):
                            row0 = h * HEADREG + LB[lvl] + g0 * 128
                            nc.sync.dma_start(
                                out=A(quad_dram, (row0, 128, 1),
                                      [[128 * 128, ng], [1, 128]]),
                                in_=A(gb, (0, 128, 1), [[256, ng], [1, 128]],
                                      off=h * 128))

            # ---------------- phase 6: gather + weighted reduce ---------------
            if KPHASE < 5:
                return
            with tc.tile_pool(name="gl", bufs=1) as gl, \
                 tc.tile_pool(name="gt", bufs=2) as gt_pool, \
                 tc.tile_pool(name="prod", bufs=1) as prod_pool, \
                 tc.tile_pool(name="red", bufs=2) as red_pool, \
                 tc.tile_pool(name="stg", bufs=2) as stg_pool, \
                 tc.tile_pool(name="gps", bufs=4, space="PSUM") as gps_pool:
                # Build the gather index stream: idx n = g*128 + p lives at
                # t16[p%16 + 16k, g*8 + p//16] for all 8 replicas k (one per
                # Q7 core pair). Cross-partition move via a phi-blocked
                # replicated DRAM read (contiguous 5KB runs), then one DVE
                # copy does the stride-8 interleave within partitions.
                t16 = gl.tile([128, T16_COLS], I16)
                TB = NSLOT_PAD // 8  # 318 g-cols per build chunk
                with tc.tile_pool(name="t16p", bufs=2) as t16p_pool:
                    for bi in range(8):
                        g0 = bi * TB
                        t16pre = t16p_pool.tile([128, 8 * TB], I16)
                        for phi in range(8):
                            nc.sync.dma_start(
                                out=t16pre[:, phi * TB:(phi + 1) * TB],
                                in_=RAW(idx_dram,
                                        [[0, 8], [NSLOT_PAD, 16], [1, TB]],
                                        off=phi * 16 * NSLOT_PAD + g0))
                        nc.vector.tensor_copy(
                            A(t16, (0, 128, 1), [[8, TB], [1, 8]], off=g0 * 8),
                            A(t16pre, (0, 128, 1), [[1, TB], [TB, 8]]))

                nstripe = 0
                for c in range(NCHUNK):
                    # valid samples in the last chunk: slots 2496-2499 only
                    nvalid = G * 128 if (c + 1) * G <= NSLOT else \
                        (NSLOT - c * G) * 128
                    gt = gt_pool.tile([128, G, 128], F32)
                    nc.gpsimd.dma_gather(
                        gt[:], quad_dram[:], t16[:, c * (G * 8):(c + 1) * (G * 8)],
                        G * 128, nvalid, 128, single_packet=True)
                    prod = prod_pool.tile([128, G * 128], F32)
                    nc.vector.tensor_tensor(
                        prod[:], gt[:],
                        A(u_t, (0, 128, 1), [[4, G], [1, 4], [0, 32]], off=c * G * 4),
                        ALU.mult)
                    red1 = red_pool.tile([128, G * 32], F32)
                    nc.vector.tensor_reduce(
                        red1[:],
                        A(prod, (0, 128, 1), [[128, G], [1, 32], [32, 4]]),
                        op=ALU.add, axis=mybir.AxisListType.X)
                    # stage in (d, g) order: stg[m, d*G + g]
                    stg = stg_pool.tile([8, 32 * G], F32)
                    for j in range(3):
                        ps = gps_pool.tile([8, QCH], F32)
                        nc.tensor.matmul(ps[:], ind8_t[:],
                                         red1[:, j * QCH:(j + 1) * QCH],
                                         start=True, stop=True)
                        nc.scalar.activation(
                            A(stg, (0, 8, 1), [[1, 16], [G, 32]], off=j * 16),
                            A(ps, (0, 8, 1), [[32, 16], [1, 32]]),
                            ACTF.Copy)
                    nc.sync.dma_start(
                        out=A(stage_dram, (0, 8, 1), [[NSLOT_PAD, 32], [1, G]],
                              off=c * G),
                        in_=A(stg, (0, 8, 1), [[G, 32], [1, G]]))
                    if debug and c == 0:
                        nc.sync.dma_start(out=dbg["d_gt"][:], in_=gt[:])
                        nc.sync.dma_start(out=dbg["d_red1"][:], in_=red1[:])

            # ---------------- phase 7: out projection -------------------------
            if KPHASE < 6:
                return
            with tc.tile_pool(name="ph7", bufs=1) as ph7, \
                 tc.tile_pool(name="p7ps", bufs=4, space="PSUM") as p7ps:
                acc_T = ph7.tile([64, NQ], F32)
                for qlo in range(4):
                    for h in range(2):
                        m = qlo * 2 + h
                        nc.sync.dma_start(
                            out=A(acc_T, (h * 32, 32, 1), [[1, NSLOT]],
                                  off=qlo * NSLOT),
                            in_=A(stage_dram, (m, 1, 1),
                                  [[NSLOT_PAD, 32], [1, NSLOT]]))
                if debug:
                    nc.sync.dma_start(out=dbg["d_acc"][:], in_=acc_T[:])
                osb = ph7.tile([128, 2, NQ], F32)
                for qc in range(NQCH):
                    q0 = qc * QCH
                    n = min(QCH, NQ - q0)
                    for mh in range(2):
                        ps = p7ps.tile([128, QCH], F32)
                        nc.tensor.matmul(ps[:, :n], wout_t[:, mh * 128:(mh + 1) * 128],
                                         acc_T[:, q0:q0 + n], start=True, stop=True)
                        nc.scalar.activation(osb[:, mh, q0:q0 + n], ps[:, :n], ACTF.Copy)
                nc.sync.dma_start(
                    out=A(out_T, (0, 128, 1), [[128 * NQ, 2], [1, NQ]]),
                    in_=osb[:])

    _body()
    nc.compile()
    return nc


def _prep_core_inputs(b, hg, query, value, reference_points,
                      W_off, b_off, W_attn, b_attn, W_val, b_val, W_out):
    """Host-side per-core input dict (all f32 numpy)."""
    f = np.float32
    vT = np.zeros((256, VT_COLS), f)
    vT[:, :NV] = value[b].T
    qT = np.ascontiguousarray(query[b].T.astype(f))

    s_arr = np.arange(32)
    h_loc = s_arr // 16
    l_arr = (s_arr // 4) % 4
    Wl = np.array([SHAPES[l][1] for l in l_arr], f)
    Hl = np.array([SHAPES[l][0] for l in l_arr], f)
    base = np.array([h_loc[s] * HEADREG + LB[l_arr[s]] for s in range(32)], f)

    boff = b_off[hg * 64:(hg + 1) * 64].astype(f)  # rows (s, xy)
    refx = np.zeros((128, NSLOT_PAD), f)
    refy = np.zeros((128, NSLOT_PAD), f)
    ref = np.asarray(reference_points)[b, :, 0, :].astype(f)  # [NQ, 2] (x, y)
    for qlo in range(4):
        rx = ref[qlo * NSLOT + np.arange(NSLOT), 0]
        ry = ref[qlo * NSLOT + np.arange(NSLOT), 1]
        for s in range(32):
            refx[qlo * 32 + s, :NSLOT] = rx * Wl[s] - 0.5 + boff[s * 2 + 0]
            refy[qlo * 32 + s, :NSLOT] = ry * Hl[s] - 0.5 + boff[s * 2 + 1]

    wvT = np.ascontiguousarray(W_val[hg * 64:(hg + 1) * 64, :].T.astype(f))
    woffT = np.ascontiguousarray(W_off[hg * 64:(hg + 1) * 64, :].T.astype(f))
    wattnT = np.ascontiguousarray(W_attn[hg * 32:(hg + 1) * 32, :].T.astype(f))
    battn = b_attn[hg * 32:(hg + 1) * 32].astype(f).reshape(32, 1)
    bvalrep = np.tile(b_val[hg * 64:(hg + 1) * 64].astype(f)[None, :], (128, 1))
    woutT = np.ascontiguousarray(W_out[:, hg * 64:(hg + 1) * 64].T.astype(f))

    p_arr = np.arange(128)
    ind8 = np.zeros((128, 8), f)
    ind8[p_arr, (p_arr // 32) * 2 + (p_arr % 32) // 16] = 1.0
    indsum = np.zeros((32, 2), f)
    indsum[np.arange(32), np.arange(32) // 16] = 1.0

    pcol = lambda v: np.ascontiguousarray(np.tile(v.astype(f), 4).reshape(128, 1))
    return {
        "vT": vT, "qT": qT, "refx": refx, "refy": refy,
        "wvT": wvT, "woffT": woffT, "wattnT": wattnT, "battn": battn,
        "bvalrep": bvalrep, "woutT": woutT, "ind8": ind8, "indsum": indsum,
        "wcol": pcol(Wl), "w2col": pcol(Wl - 2), "h2col": pcol(Hl - 2),
        "basecol": pcol(base),
    }


def run_cores(inputs, debug=False, trace=False):
    key = ("nc", debug)
    if key not in _CACHE:
        _CACHE[key] = build_nc(debug=debug)
    nc = _CACHE[key]
    in_maps = [_prep_core_inputs(c // 4, c % 4, **inputs) for c in range(8)]
    res = bass_utils.run_bass_kernel_spmd(nc, in_maps, core_ids=list(range(8)),
                                          trace=trace)
    return res


def kernel(query, value, reference_points, spatial_shapes,
           W_off, b_off, W_attn, b_attn, W_val, b_val, W_out, b_out,
           _debug=False, _trace=False):
    inputs = dict(query=np.asarray(query), value=np.asarray(value),
                  reference_points=np.asarray(reference_points),
                  W_off=np.asarray(W_off), b_off=np.asarray(b_off),
                  W_attn=np.asarray(W_attn), b_attn=np.asarray(b_attn),
                  W_val=np.asarray(W_val), b_val=np.asarray(b_val),
                  W_out=np.asarray(W_out))
    res = run_cores(inputs, debug=_debug, trace=_trace)
    out = np.zeros((BS, NQ, 256), np.float32)
    for b in range(BS):
        acc = np.zeros((256, NQ), np.float32)
        for hg in range(4):
            acc += res.results[b * 4 + hg]["out_T"]
        out[b] = acc.T + np.asarray(b_out)[None, :].astype(np.float32)
    kernel._last_res = res
    return out
